# revision 8
# baseline (speedup 1.0000x reference)
"""Trainium2 Bass kernel for nn_FocalToVoxelNeXtBridge — v2 (scatter-free).

Pipeline (per NeuronCore, 8 cores = batch(2) x y-strip(4)):
  The host lays xT out in DENSE CELL ORDER: per 10-row band, 5120 columns
  (one per BEV cell; zeros for inactive cells, the rank-0 voxel of each
  active cell otherwise) plus chunk-aligned 128-column appendix groups
  holding the rank>=1 (duplicate) voxels of each 512-cell chunk.

  1. proj (weight-stationary): psum[C, x] = w1.T @ xT_chunk gives the
     dense BEV band rows DIRECTLY in conv rhs layout.  Zero columns
     project to relu(0)=0, so inactive cells are correct with no scatter,
     no memset, no dense-HBM roundtrip, no transposes.
  2. dups: appendix groups are projected token-major (data-stationary),
     relu'd, then folded into the band psum with one matmul against an
     on-device 0/1 expansion matrix E[tok, x] = (cell(tok) == x), built
     by a single iota + is_equal tensor_scalar per group.
  3. conv: 3x3 subm conv as 9 shifted matmuls per output row over the
     in-SBUF band rows, + K=1 "penalty" matmul adding -1e30 at inactive
     cells so the final ReLU zeroes them (subm mask).  BN2 folded into
     conv weights, shift via ACT bias.  Output rows (y, C, X) in bf16;
     host transposes and widens to f32.
"""

import os

import numpy as np
import ml_dtypes

BF16 = ml_dtypes.bfloat16

B, Y, X, C, CIN = 2, 512, 512, 128, 192
N = 400000
EPS1, EPS2 = 1e-5, 1e-3
STRIPS = 4
SH = Y // STRIPS          # 128 output rows per core
HLOC = SH + 2             # dense rows incl. +-1 halo
BAND_ROWS = 10
NBANDS = HLOC // BAND_ROWS            # 13
BCELLS = BAND_ROWS * X                # 5120 cells per band
NCHUNK = BCELLS // 512                # 10 chunks of 512 cells
NEG = -1e30

_PROG_CACHE: dict = {}
LAST_EXEC_NS = None
LAST_RESULTS = None


# ----------------------------------------------------------------- host plan

def _plan_core(bi, yi, xi, b, s):
    """Voxels of one core sorted by local cell; returns vox ids, local cell
    (0..HLOC*X) and that cell's voxel count."""
    y0 = s * SH
    lo = y0 - 1
    m = (bi == b) & (yi >= lo) & (yi <= y0 + SH)
    vox = np.nonzero(m)[0]
    cell = (yi[vox] - lo).astype(np.int64) * X + xi[vox]
    order = np.argsort(cell, kind="stable")
    vox, cell = vox[order], cell[order]
    uniq, inv, counts = np.unique(cell, return_inverse=True,
                                  return_counts=True)
    return vox, cell, counts[inv]


# ------------------------------------------------------------- device program

def _build_program(capg):
    """capg[j][c] = number of 128-token appendix groups for band j chunk c
    (uniform across cores)."""
    import concourse.bacc as bacc
    import concourse.mybir as mybir
    import concourse.tile as tile

    dt = mybir.dt
    ngb = capg.sum(axis=1)                    # appendix groups per band
    Wb = BCELLS + 128 * ngb                   # xT columns per band
    band_off = np.concatenate([[0], np.cumsum(Wb)])[:-1]
    NG = int(ngb.sum())                       # total appendix groups
    goff = np.concatenate([[0], np.cumsum(capg.flatten())])[:-1].reshape(capg.shape)
    TOT = int(Wb.sum())

    nc = bacc.Bacc("TRN2", target_bir_lowering=False, debug=False)

    h_xT = nc.dram_tensor("xT", [CIN + 1, TOT], dt.bfloat16, kind="ExternalInput")
    h_w1 = nc.dram_tensor("w1", [CIN + 1, C], dt.bfloat16, kind="ExternalInput")
    h_cw = nc.dram_tensor("convw", [9, C, C], dt.bfloat16, kind="ExternalInput")
    h_ones = nc.dram_tensor("onesw", [1, C], dt.bfloat16, kind="ExternalInput")
    h_b2 = nc.dram_tensor("bias2", [C, 1], dt.float32, kind="ExternalInput")
    h_pen = nc.dram_tensor("pen", [1, SH * X], dt.bfloat16, kind="ExternalInput")
    h_cr = nc.dram_tensor("cellrel", [128, max(NG, 1)], dt.float32,
                          kind="ExternalInput")
    h_out = nc.dram_tensor("out_t", [SH, C, X], dt.bfloat16, kind="ExternalOutput")

    with tile.TileContext(nc) as tc:
        with (
            tc.tile_pool(name="const", bufs=1) as wp,
            tc.tile_pool(name="xa", bufs=3) as xap,
            tc.tile_pool(name="xb", bufs=3) as xbp,
            tc.tile_pool(name="rows", bufs=4) as rp,
            tc.tile_pool(name="fd", bufs=3) as fdp,
            tc.tile_pool(name="ep", bufs=3) as epp,
            tc.tile_pool(name="osb", bufs=6) as op,
            tc.tile_pool(name="penp", bufs=4) as pnp,
            tc.tile_pool(name="pp", bufs=4, space="PSUM") as pp,
            tc.tile_pool(name="cp", bufs=2, space="PSUM") as cp,
        ):
            # ---- band tiles (declared early so band 0/1 loads lead)
            rows_t = [None] * NBANDS
            fbd_t = [None] * NBANDS
            xt_t = {}

            w1a = wp.tile([128, C], dt.bfloat16)
            w1b = wp.tile([CIN + 1 - 128, C], dt.bfloat16)
            nc.sync.dma_start(out=w1a[:], in_=h_w1[0:128, :])
            nc.sync.dma_start(out=w1b[:], in_=h_w1[128:, :])
            def load_band(j):
                cap = int(Wb[j])
                c0 = int(band_off[j])
                xa = xap.tile([128, cap], dt.bfloat16, tag="xa", name=f"xa{j}")
                xb = xbp.tile([CIN + 1 - 128, cap], dt.bfloat16, tag="xb",
                              name=f"xb{j}")
                nc.sync.dma_start(out=xa[:], in_=h_xT[0:128, c0:c0 + cap])
                nc.gpsimd.dma_start(out=xb[:], in_=h_xT[128:, c0:c0 + cap])
                return xa, xb
            xt_t[0] = load_band(0)
            xt_t[1] = load_band(1)
            wconv = wp.tile([C, 9 * C], dt.bfloat16)
            for t in range(9):
                nc.sync.dma_start(out=wconv[:, C * t:C * (t + 1)], in_=h_cw[t])
            ones = wp.tile([1, C], dt.bfloat16)
            nc.sync.dma_start(out=ones[:], in_=h_ones[:])
            b2 = wp.tile([C, 1], dt.float32)
            nc.sync.dma_start(out=b2[:], in_=h_b2[:])
            cellrel = wp.tile([128, max(NG, 1)], dt.float32)
            nc.sync.dma_start(out=cellrel[:], in_=h_cr[:])
            iota16 = wp.tile([128, 512], dt.int16)
            nc.gpsimd.iota(iota16[:], pattern=[[1, 512]], base=0,
                           channel_multiplier=0)

            def appendix_proj(j):
                """Token-major proj + relu of band j's appendix groups."""
                ng = int(ngb[j])
                if ng == 0:
                    fbd_t[j] = None
                    return
                xa, xb = xt_t[j]
                fbd = fdp.tile([128, ng * C], dt.bfloat16, tag="fd",
                               name=f"fd{j}")
                for g0 in range(0, ng, 4):
                    gw = min(4, ng - g0)
                    pd = pp.tile([128, 512], dt.float32, tag="ps",
                                 name=f"pd{j}_{g0}")
                    for g in range(g0, g0 + gw):
                        col = BCELLS + g * 128
                        o = (g - g0) * 128
                        # one start per PSUM bank (2KB zero region), one
                        # stop on the bank's final matmul
                        nc.tensor.matmul(pd[:, o:o + 128],
                                         xa[:, col:col + 128], w1a[:],
                                         start=(g == g0), stop=False)
                        nc.tensor.matmul(pd[:, o:o + 128],
                                         xb[:, col:col + 128], w1b[:],
                                         start=False, stop=(g == g0 + gw - 1))
                    if (g0 // 4) % 2 == 0:
                        nc.vector.tensor_relu(
                            out=fbd[:, g0 * C:(g0 + gw) * C],
                            in_=pd[:, 0:gw * 128])
                    else:
                        nc.scalar.activation(
                            fbd[:, g0 * C:(g0 + gw) * C], pd[:, 0:gw * 128],
                            mybir.ActivationFunctionType.Relu)
                fbd_t[j] = fbd

            def dense_proj(j):
                lim = 10 * (j - 1) + 6
                """Weight-stationary proj of band j's dense columns, with
                appendix fold via E-matmuls, relu into rows_t[j]."""
                xa, xb = xt_t[j]
                fbd = fbd_t[j]
                rt = rp.tile([128, BCELLS], dt.bfloat16, tag="rows",
                             name=f"rows{j}")
                for c in range(NCHUNK):
                    o = c * 512
                    ps = pp.tile([128, 512], dt.float32, tag="ps",
                                 name=f"ps{j}_{c}")
                    ngc = int(capg[j, c])
                    nc.tensor.matmul(ps[:], w1a[:], xa[:, o:o + 512],
                                     start=True, stop=False)
                    nc.tensor.matmul(ps[:], w1b[:], xb[:, o:o + 512],
                                     start=False, stop=(ngc == 0))
                    for k in range(ngc):
                        g = int(goff[j, c]) - int(goff[j, 0]) + k
                        gidx = int(goff[j, c]) + k
                        E = epp.tile([128, 512], dt.bfloat16, tag="E")
                        nc.vector.tensor_scalar(
                            out=E[:], in0=iota16[:],
                            scalar1=cellrel[:, gidx:gidx + 1], scalar2=None,
                            op0=mybir.AluOpType.is_equal)
                        nc.tensor.matmul(ps[:], fbd[:, g * C:(g + 1) * C],
                                         E[:], start=False, stop=(k == ngc - 1))
                    if c % 2 == 0:
                        nc.vector.tensor_relu(out=rt[:, o:o + 512], in_=ps[:])
                    else:
                        nc.scalar.activation(
                            rt[:, o:o + 512], ps[:],
                            mybir.ActivationFunctionType.Relu)
                    if next_g0[0] < SH and next_g0[0] <= lim:
                        emit_group(next_g0[0])
                        next_g0[0] += 2
                rows_t[j] = rt

            # ---- conv emission (groups of 2 output rows)
            TAPS = [(1, 1), (0, 1), (2, 1), (0, 0), (0, 2), (1, 0), (1, 2),
                    (2, 0), (2, 2)]

            def row_slice(L, lo, hi):
                """AP for dense row L columns [lo,hi) in its band tile."""
                j, r = L // BAND_ROWS, L % BAND_ROWS
                base = r * X
                return rows_t[j][:, base + lo:base + hi]

            def emit_group(g0):
                ys = (g0, g0 + 1)
                pst = cp.tile([128, 2 * X], dt.float32, tag="cps",
                              name=f"cps{g0}")
                peng = pnp.tile([1, 2 * X], dt.bfloat16, tag="pen")
                nc.gpsimd.dma_start(out=peng[:],
                                    in_=h_pen[0:1, g0 * X:(g0 + 2) * X])
                for dy, dx in TAPS:
                    w = wconv[:, C * (dy * 3 + dx):C * (dy * 3 + dx + 1)]
                    for y in ys:
                        o = (y - g0) * X
                        L = y + dy
                        if dx == 1:
                            nc.tensor.matmul(pst[:, o:o + X], w,
                                             row_slice(L, 0, X),
                                             start=(dy == 1), stop=False)
                        elif dx == 0:
                            nc.tensor.matmul(pst[:, o + 1:o + X], w,
                                             row_slice(L, 0, X - 1),
                                             start=False, stop=False)
                        else:
                            nc.tensor.matmul(pst[:, o:o + X - 1], w,
                                             row_slice(L, 1, X),
                                             start=False, stop=False)
                for y in ys:
                    o = (y - g0) * X
                    # each row's X-slice is exactly one PSUM bank; stop
                    # closes that bank's accumulation group (sim-only flag)
                    nc.tensor.matmul(pst[:, o:o + X], ones[:],
                                     peng[0:1, o:o + X],
                                     start=False, stop=True)
                osb = op.tile([128, 2 * X], dt.bfloat16, tag="osb",
                              name=f"osb{g0}")
                if (g0 // 2) % 2 == 0:
                    nc.scalar.activation(osb[:], pst[:],
                                         mybir.ActivationFunctionType.Relu,
                                         bias=b2[:, 0:1])
                else:
                    nc.vector.tensor_scalar(
                        out=osb[:], in0=pst[:], scalar1=b2[:, 0:1],
                        scalar2=0.0, op0=mybir.AluOpType.add,
                        op1=mybir.AluOpType.max)
                nc.sync.dma_start(out=h_out[g0], in_=osb[:, 0:X])
                nc.gpsimd.dma_start(out=h_out[g0 + 1], in_=osb[:, X:2 * X])

            next_g0 = [0]

            def emit_conv_up_to(limit):
                while next_g0[0] < SH and next_g0[0] <= limit:
                    emit_group(next_g0[0])
                    next_g0[0] += 2

            # ---- main pipeline
            appendix_proj(0)
            for j in range(NBANDS):
                if j + 2 < NBANDS:
                    xt_t[j + 2] = load_band(j + 2)
                if j + 1 < NBANDS:
                    appendix_proj(j + 1)
                dense_proj(j)
                # groups needing rows <= 10(j-1)+9 (conv group g0 reads
                # dense rows g0..g0+3)
                emit_conv_up_to(10 * (j - 1) + 6)
            emit_conv_up_to(SH - 2)
    nc.finalize()
    return nc


# ------------------------------------------------------------------ execution

def _ensure_ntff_hook():
    import sys
    import types
    try:
        from antenv.axon_hooks import get_axon_ntff_profile_hook  # noqa: F401
        return
    except ImportError:
        pass
    try:
        import antenv
        from trn_agent_boot.trn_boot import _ntff_profile_via_ctypes
        mod = types.ModuleType("antenv.axon_hooks")
        state = {"h": None}
        mod.set_axon_ntff_profile_hook = lambda h: state.__setitem__("h", h)
        mod.get_axon_ntff_profile_hook = lambda: state["h"]
        sys.modules["antenv.axon_hooks"] = mod
        antenv.axon_hooks = mod
        mod.set_axon_ntff_profile_hook(
            _ntff_profile_via_ctypes("/opt/axon/libaxon_pjrt.so"))
    except Exception as e:  # pragma: no cover
        print(f"ntff hook setup failed: {e}")


def _host_prep(inputs):
    vf = np.asarray(inputs["voxel_features"], np.float32)
    vc = np.asarray(inputs["voxel_coords"], np.int32)
    W_proj = np.asarray(inputs["W_proj"], np.float32)
    b_proj = np.asarray(inputs["b_proj"], np.float32)
    g1 = np.asarray(inputs["bn1_gamma"], np.float32)
    be1 = np.asarray(inputs["bn1_beta"], np.float32)
    mu1 = np.asarray(inputs["bn1_mean"], np.float32)
    v1 = np.asarray(inputs["bn1_var"], np.float32)
    conv_w = np.asarray(inputs["conv_w"], np.float32)
    conv_b = np.asarray(inputs["conv_b"], np.float32)
    g2 = np.asarray(inputs["bn2_gamma"], np.float32)
    be2 = np.asarray(inputs["bn2_beta"], np.float32)
    mu2 = np.asarray(inputs["bn2_mean"], np.float32)
    v2 = np.asarray(inputs["bn2_var"], np.float32)

    s1 = g1 / np.sqrt(v1 + EPS1)
    t1 = (b_proj - mu1) * s1 + be1
    w1 = np.concatenate([W_proj * s1[None, :], t1[None, :]], 0)  # (193,128)
    s2 = g2 / np.sqrt(v2 + EPS2)
    t2 = (conv_b - mu2) * s2 + be2
    cw = (conv_w * s2[None, None, None, :]).reshape(9, C, C)

    bi, yi, xi = vc[:, 0], vc[:, 2], vc[:, 3]
    active = np.zeros((B, Y, X), bool)
    active[bi, yi, xi] = True

    plans = []
    for core in range(8):
        b, s = core // STRIPS, core % STRIPS
        plans.append(_plan_core(bi, yi, xi, b, s))

    # uniform appendix group capacities: capg[j][c].  The appendix holds
    # ALL voxels of multi-voxel cells (relu per voxel, then E-matmul sum).
    capg = np.zeros((NBANDS, NCHUNK), np.int64)
    for vox, cell, cnts in plans:
        dcell = cell[cnts >= 2]
        j = dcell // BCELLS
        c = (dcell % BCELLS) // 512
        for jj in range(NBANDS):
            cnt = np.bincount(c[j == jj], minlength=NCHUNK)
            capg[jj] = np.maximum(capg[jj], (cnt + 127) // 128)
    return dict(vf=vf, w1=w1, cw=cw, t2=t2, active=active, plans=plans,
                capg=capg)


def _build_inputs(prep):
    vf, w1, cw, t2 = prep["vf"], prep["w1"], prep["cw"], prep["t2"]
    active, plans, capg = prep["active"], prep["plans"], prep["capg"]

    ngb = capg.sum(axis=1)
    Wb = BCELLS + 128 * ngb
    band_off = np.concatenate([[0], np.cumsum(Wb)])[:-1]
    goff = np.concatenate([[0], np.cumsum(capg.flatten())])[:-1].reshape(capg.shape)
    NG = int(ngb.sum())
    TOT = int(Wb.sum())

    onesw = np.ones((1, C), BF16)
    w1_b = w1.astype(BF16)
    cw_b = cw.astype(BF16)
    b2_h = t2.reshape(C, 1).astype(np.float32)

    in_maps = []
    for core in range(8):
        b, s = core // STRIPS, core % STRIPS
        vox, cell, cnts = plans[core]
        xT = np.zeros((CIN + 1, TOT), BF16)
        cr = np.full((128, max(NG, 1)), -1, np.float32)

        j = cell // BCELLS
        cellin = cell - j * BCELLS
        # dense columns: single-voxel cells only
        m0 = cnts == 1
        col0 = band_off[j[m0]] + cellin[m0]
        xT[:CIN, col0] = vf[vox[m0]].T.astype(BF16)
        xT[CIN, col0] = np.ones(int(m0.sum()), BF16)
        # appendix: every voxel of multi-voxel cells, per (band, chunk)
        md = cnts >= 2
        dj, dcell, dvox = j[md], cellin[md], vox[md]
        dc = dcell // 512
        for jj in range(NBANDS):
            for cc in range(NCHUNK):
                sel = (dj == jj) & (dc == cc)
                n = int(sel.sum())
                if n == 0:
                    continue
                base = band_off[jj] + BCELLS + (goff[jj, cc] - goff[jj, 0]) * 128
                cols = base + np.arange(n)
                xT[:CIN, cols] = vf[dvox[sel]].T.astype(BF16)
                xT[CIN, cols] = np.ones(n, BF16)
                rel = (dcell[sel] - 512 * cc).astype(np.float32)
                for g in range(int(capg[jj, cc])):
                    gi = goff[jj, cc] + g
                    part = rel[g * 128:(g + 1) * 128]
                    cr[0:len(part), gi] = part
        pena = np.where(active[b, s * SH:(s + 1) * SH], 0.0, NEG)
        pen = pena.reshape(1, SH * X).astype(BF16)
        in_maps.append(dict(
            xT=np.ascontiguousarray(xT), w1=w1_b, convw=cw_b, onesw=onesw,
            bias2=b2_h, pen=np.ascontiguousarray(pen),
            cellrel=np.ascontiguousarray(cr)))
    return in_maps


def kernel(**inputs):
    global LAST_EXEC_NS, LAST_RESULTS
    prep = _host_prep(inputs)
    in_maps = _build_inputs(prep)
    capg = prep["capg"]

    key = tuple(capg.flatten().tolist())
    if key not in _PROG_CACHE:
        _PROG_CACHE[key] = _build_program(capg)
    nc = _PROG_CACHE[key]

    from concourse.bass_utils import run_bass_kernel_spmd
    trace = os.environ.get("KERNEL_TRACE", "0") == "1"
    if trace:
        _ensure_ntff_hook()
    res = run_bass_kernel_spmd(nc, in_maps, core_ids=list(range(8)),
                               trace=trace)
    LAST_EXEC_NS = res.exec_time_ns
    LAST_RESULTS = res

    out = np.empty((B, Y, X, C), np.float32)
    for core in range(8):
        b, s = core // STRIPS, core % STRIPS
        r = np.asarray(res.results[core]["out_t"])  # (SH, C, X) bf16
        out[b, s * SH:(s + 1) * SH] = r.transpose(0, 2, 1).astype(np.float32)
    return out
